# revision 3
# baseline (speedup 1.0000x reference)
"""Trainium2 Bass kernel for MultiHeadHypergraphAttention.

Problem: queries (4, 1024, 512), keys (4, 4096, 512), incidence (4, 1024, 4096) i32,
torch-Linear Q/K/V/O projections, per-head masked softmax attention.

Sharding (8 cores): batch (4) x head-group (2 groups of 4 heads).
Core c handles batch b = c//2, head group g = c%2 and produces the partial
output projection for its 4 heads; the host sums the two partials per batch.

Device-side layout ("scores transposed"): S^T is computed with nodes on
partitions and edges on the free axis, so attention-weight blocks are directly
usable as matmul stationary operands for attn@V, and the incidence mask
(host-transposed to (nodes, edges)) is applied in its natural layout.
Softmax normalization is folded into the output: V is augmented with a
ones-column so the attn@V matmul also produces row sums; attention outputs
are divided by those sums afterwards (masked entries are exactly
exp(-40 + s/8) ~ 1e-16, i.e. zero in bf16/f32 sums).

All matmuls run in bf16 (1 cycle/row on TRN2 PE) with f32 PSUM accumulation.
Head 0 of each group applies the mask additively in PSUM via a 320-scaled
identity matmul (exp bias -40); the other heads multiply exp(S)/8 by the
bf16 mask on the vector engine — this balances PE/ACT/DVE load.
"""

import sys
import os

for _p in ("/opt/trn_rl_repo",):
    if _p not in sys.path and os.path.isdir(_p):
        sys.path.insert(0, _p)

import numpy as np
from contextlib import ExitStack

import concourse.bass as bass
import concourse.mybir as mybir
import concourse.tile as tile
from concourse import bacc
from concourse.bass_utils import run_bass_kernel_spmd

BF16 = mybir.dt.bfloat16
F32 = mybir.dt.float32
I32 = mybir.dt.int32

BS, E, N, D = 4, 1024, 4096, 512
H, DK = 8, 64            # global heads
HL = 4                   # heads per core (local)
NCHUNK = N // 128        # 32
ECHUNK = E // 128        # 8
C_MASK = 320.0           # identity scale for additive mask (320 * 0.125 = 40)
EXP_BIAS = -40.0

LAST_EXEC_TIME_NS = None
_CACHED_NC = None


def _build_nc():
    nc = bacc.Bacc("TRN2", target_bir_lowering=False, debug=False, num_devices=8)

    qT_d = nc.dram_tensor("qT", (D, E), F32, kind="ExternalInput").ap()
    kT_d = nc.dram_tensor("kT", (D, N), F32, kind="ExternalInput").ap()
    mT_d = nc.dram_tensor("mT", (N, E), I32, kind="ExternalInput").ap()
    wqT_d = nc.dram_tensor("wqT", (D, 256), F32, kind="ExternalInput").ap()
    wkT_d = nc.dram_tensor("wkT", (D, 256), F32, kind="ExternalInput").ap()
    wvT_d = nc.dram_tensor("wvT", (D + 1, 260), F32, kind="ExternalInput").ap()
    woT_d = nc.dram_tensor("woT", (2, 128, 512), F32, kind="ExternalInput").ap()
    bq_d = nc.dram_tensor("bq2", (2, 128, 1), F32, kind="ExternalInput").ap()
    bk_d = nc.dram_tensor("bk2", (2, 128, 1), F32, kind="ExternalInput").ap()
    bo_d = nc.dram_tensor("bo_row", (1, 512), F32, kind="ExternalInput").ap()
    out_d = nc.dram_tensor("out", (E, 512), F32, kind="ExternalOutput").ap()

    with tile.TileContext(nc) as tc, ExitStack() as ctx:
        persist = ctx.enter_context(tc.tile_pool(name="persist", bufs=1))
        work = ctx.enter_context(tc.tile_pool(name="work", bufs=1))
        ps = ctx.enter_context(tc.tile_pool(name="ps", bufs=1, space="PSUM"))

        # ---------------- constants ----------------
        ones_row = persist.tile([1, 128], BF16, tag="ones_row")
        nc.vector.memset(ones_row, 1.0)
        bias_m40 = persist.tile([128, 1], F32, tag="bias_m40")
        nc.vector.memset(bias_m40, EXP_BIAS)

        def make_ident(tag, fill):
            t = persist.tile([128, 128], BF16, tag=tag, name=tag)
            nc.gpsimd.memset(t, 0.0)
            nc.gpsimd.affine_select(
                out=t, in_=t, compare_op=mybir.AluOpType.not_equal,
                fill=fill, base=0, pattern=[[-1, 128]], channel_multiplier=1)
            return t

        ident_mask = make_ident("ident_mask", C_MASK)
        ident_one = make_ident("ident_one", 1.0)

        # ---------------- weight / input loads (DMA casts f32->bf16) -------
        def load_cast(tag, dram_ap, shape):
            t = persist.tile(list(shape), BF16, tag=tag, name=tag)
            nc.gpsimd.dma_start(out=t, in_=dram_ap)
            return t

        qTb = [load_cast(f"qTb{c}", qT_d[c * 128:(c + 1) * 128, :], (128, E))
               for c in range(4)]
        kTb = [load_cast(f"kTb{c}", kT_d[c * 128:(c + 1) * 128, :], (128, N))
               for c in range(4)]
        wqTb = [load_cast(f"wqTb{c}", wqT_d[c * 128:(c + 1) * 128, :], (128, 256))
                for c in range(4)]
        wkTb = [load_cast(f"wkTb{c}", wkT_d[c * 128:(c + 1) * 128, :], (128, 256))
                for c in range(4)]
        wvTb = [load_cast(f"wvTb{c}", wvT_d[c * 128:(c + 1) * 128, :], (128, 260))
                for c in range(4)]
        wv_bias = load_cast("wv_bias", wvT_d[D:D + 1, :], (1, 260))
        woTb = [load_cast(f"woTb{p}", woT_d[p], (128, 512)) for p in range(2)]
        bo_row = load_cast("bo_row", bo_d, (1, 512))
        bqs = []
        bks = []
        for p in range(2):
            bq_t = persist.tile([128, 1], F32, tag=f"bq{p}", name=f"bq{p}")
            nc.sync.dma_start(out=bq_t, in_=bq_d[p])
            bqs.append(bq_t)
            bk_t = persist.tile([128, 1], F32, tag=f"bk{p}", name=f"bk{p}")
            nc.sync.dma_start(out=bk_t, in_=bk_d[p])
            bks.append(bk_t)

        # ---------------- phase A: projections ----------------
        # QT[p] (128, 1024): rows = local dims [p*128, (p+1)*128), cols = edges
        QTs = [persist.tile([128, E], BF16, tag=f"QTs{p}", name=f"QTs{p}")
               for p in range(2)]
        for p in range(2):
            qp = ps.tile([128, E], F32, tag="st", bufs=2)
            for c in range(4):
                for e2 in range(2):
                    nc.tensor.matmul(
                        qp[:, e2 * 512:(e2 + 1) * 512],
                        wqTb[c][:, p * 128:(p + 1) * 128],
                        qTb[c][:, e2 * 512:(e2 + 1) * 512],
                        start=(c == 0), stop=(c == 3))
            nc.vector.tensor_scalar_add(QTs[p], qp, bqs[p])

        # KT[p] (128, 4096)
        KTs = [persist.tile([128, N], BF16, tag=f"KTs{p}", name=f"KTs{p}")
               for p in range(2)]
        for p in range(2):
            for nw in range(8):
                kp = ps.tile([128, 512], F32, tag="bankA", bufs=2)
                for c in range(4):
                    nc.tensor.matmul(
                        kp, wkTb[c][:, p * 128:(p + 1) * 128],
                        kTb[c][:, nw * 512:(nw + 1) * 512],
                        start=(c == 0), stop=(c == 3))
                nc.vector.tensor_scalar_add(
                    KTs[p][:, nw * 512:(nw + 1) * 512], kp, bks[p])

        # V' (4096, 260) as (128, 32*260); per node-chunk cols [n*260, +260)
        # col layout within a chunk: head l in [l*65, l*65+64) = V, l*65+64 = ones
        Vs = persist.tile([128, NCHUNK * 260], BF16, tag="Vs")
        for n in range(NCHUNK):
            vp = ps.tile([128, 260], F32, tag="bankB", bufs=2)
            for c in range(4):
                nc.tensor.matmul(vp, kTb[c][:, n * 128:(n + 1) * 128], wvTb[c],
                                 start=(c == 0), stop=False)
            nc.tensor.matmul(vp, ones_row, wv_bias, start=False, stop=True)
            nc.vector.tensor_copy(Vs[:, n * 260:(n + 1) * 260], vp)

        # resident bf16 transposed mask (nodes on partitions)
        Mb = persist.tile([128, NCHUNK * E], BF16, tag="Mb")

        # ---------------- phase B: attention ----------------
        pairT_sb = [persist.tile([128, E], BF16, tag=f"pairT{p}", name=f"pairT{p}")
                    for p in range(2)]
        for l in range(HL):
            p, r = l // 2, l % 2
            accA = ps.tile([128, 512], F32, tag="bankA", bufs=2, name=f"accA{l}")
            accB = ps.tile([128, 512], F32, tag="bankA", bufs=2, name=f"accB{l}")
            if r == 0:
                pairT_ps = ps.tile([128, E], BF16, tag="bankB", bufs=2,
                                   name=f"pairT_ps{p}")
            kslice = KTs[p][64 * r:64 * r + 64, :]
            qslice = QTs[p][64 * r:64 * r + 64, :]
            for n in range(NCHUNK):
                if l == 0:
                    nc.gpsimd.dma_start(
                        out=Mb[:, n * E:(n + 1) * E],
                        in_=mT_d[n * 128:(n + 1) * 128, :])
                st = ps.tile([128, E], F32, tag="st", bufs=2, name=f"st{l}_{n}")
                if l == 0:
                    for e2 in range(2):
                        sl = slice(e2 * 512, (e2 + 1) * 512)
                        nc.tensor.matmul(st[:, sl], ident_mask,
                                         Mb[:, n * E + e2 * 512:n * E + (e2 + 1) * 512],
                                         start=True, stop=False)
                        nc.tensor.matmul(st[:, sl],
                                         kslice[:, n * 128:(n + 1) * 128],
                                         qslice[:, sl], start=False, stop=True)
                    P = work.tile([128, E], BF16, tag="P", bufs=3,
                                  name=f"P{l}_{n}")
                    nc.scalar.activation(P, st, mybir.ActivationFunctionType.Exp,
                                         bias=bias_m40, scale=0.125)
                else:
                    for e2 in range(2):
                        sl = slice(e2 * 512, (e2 + 1) * 512)
                        nc.tensor.matmul(st[:, sl],
                                         kslice[:, n * 128:(n + 1) * 128],
                                         qslice[:, sl], start=True, stop=True)
                    Praw = work.tile([128, E], BF16, tag="Praw", bufs=3,
                                     name=f"Praw{l}_{n}")
                    nc.scalar.activation(Praw, st,
                                         mybir.ActivationFunctionType.Exp,
                                         bias=0.0, scale=0.125)
                    P = work.tile([128, E], BF16, tag="P", bufs=3,
                                  name=f"P{l}_{n}")
                    nc.vector.tensor_mul(P, Praw, Mb[:, n * E:(n + 1) * E])
                for e in range(ECHUNK):
                    acc = accA if e < 4 else accB
                    off = (e % 4) * 128
                    nc.tensor.matmul(
                        acc[:, off:off + 65],
                        P[:, e * 128:(e + 1) * 128],
                        Vs[:, n * 260 + l * 65:n * 260 + l * 65 + 65],
                        start=(n == 0 and e % 4 == 0),
                        stop=(n == NCHUNK - 1 and e % 4 == 3))
            # normalize by row sums and transpose into pair tile
            for e in range(ECHUNK):
                acc = accA if e < 4 else accB
                off = (e % 4) * 128
                recip = work.tile([128, 1], F32, tag="recip", bufs=4,
                                  name=f"recip{l}_{e}")
                nc.vector.reciprocal(recip, acc[:, off + 64:off + 65])
                normed = work.tile([128, 64], BF16, tag="normed", bufs=4,
                                   name=f"normed{l}_{e}")
                nc.vector.tensor_scalar_mul(normed, acc[:, off:off + 64], recip)
                nc.tensor.transpose(
                    pairT_ps[64 * r:64 * r + 64, e * 128:(e + 1) * 128],
                    normed, ident_one)
            if r == 1:
                nc.vector.tensor_copy(pairT_sb[p], pairT_ps)

        # ---------------- phase C: output projection (partial) -------------
        for e in range(ECHUNK):
            f = ps.tile([128, 512], F32, tag="bankB", bufs=2, name=f"fin{e}")
            nc.tensor.matmul(f, pairT_sb[0][:, e * 128:(e + 1) * 128], woTb[0],
                             start=True, stop=False)
            nc.tensor.matmul(f, pairT_sb[1][:, e * 128:(e + 1) * 128], woTb[1],
                             start=False, stop=False)
            nc.tensor.matmul(f, ones_row, bo_row, start=False, stop=True)
            fo = work.tile([128, 512], F32, tag="fo", bufs=2, name=f"fo{e}")
            nc.vector.tensor_copy(fo, f)
            nc.sync.dma_start(out=out_d[e * 128:(e + 1) * 128, :], in_=fo)

    nc.compile()
    return nc


def _get_nc():
    global _CACHED_NC
    if _CACHED_NC is None:
        _CACHED_NC = _build_nc()
    return _CACHED_NC


def _make_in_maps(queries, keys, incidence_matrix, Wq, bq, Wk, bk, Wv, bv, Wo, bo):
    """Host-side sharding + layout marshalling (transposes only)."""
    queries = np.asarray(queries, dtype=np.float32)
    keys = np.asarray(keys, dtype=np.float32)
    incidence = np.ascontiguousarray(np.asarray(incidence_matrix, dtype=np.int32))
    Wq = np.asarray(Wq, dtype=np.float32)
    Wk = np.asarray(Wk, dtype=np.float32)
    Wv = np.asarray(Wv, dtype=np.float32)
    Wo = np.asarray(Wo, dtype=np.float32)
    bq = np.asarray(bq, dtype=np.float32)
    bk = np.asarray(bk, dtype=np.float32)
    bv = np.asarray(bv, dtype=np.float32)
    bo = np.asarray(bo, dtype=np.float32)

    in_maps = []
    for core in range(8):
        b, g = core // 2, core % 2
        sl = slice(g * 256, (g + 1) * 256)
        wvT = np.zeros((D + 1, 260), np.float32)
        for l in range(HL):
            rows = slice(g * 256 + l * 64, g * 256 + l * 64 + 64)
            wvT[:D, l * 65:l * 65 + 64] = Wv[rows, :].T
            wvT[D, l * 65:l * 65 + 64] = bv[rows]
            wvT[D, l * 65 + 64] = 1.0
        in_maps.append({
            "qT": np.ascontiguousarray(queries[b].T),
            "kT": np.ascontiguousarray(keys[b].T),
            "mT": np.ascontiguousarray(incidence[b].T),
            "wqT": np.ascontiguousarray(Wq[sl, :].T),
            "wkT": np.ascontiguousarray(Wk[sl, :].T),
            "wvT": wvT,
            "woT": np.ascontiguousarray(Wo[:, sl].T).reshape(2, 128, 512).copy(),
            "bq2": bq[sl].reshape(2, 128, 1).copy(),
            "bk2": bk[sl].reshape(2, 128, 1).copy(),
            "bo_row": (bo[None, :] if g == 0 else np.zeros((1, 512), np.float32)).copy(),
        })
    return in_maps


def kernel(**inputs):
    global LAST_EXEC_TIME_NS
    nc = _get_nc()
    in_maps = _make_in_maps(**inputs)
    trace = bool(os.environ.get("BASS_TRACE"))
    if trace:
        _install_ntff_hook()
    res = run_bass_kernel_spmd(nc, in_maps, core_ids=list(range(8)), trace=trace)
    LAST_EXEC_TIME_NS = res.exec_time_ns
    out = np.zeros((BS, E, D), np.float32)
    for b in range(BS):
        out[b] = res.results[2 * b]["out"] + res.results[2 * b + 1]["out"]
    return out


def _install_ntff_hook():
    """Recreate the missing antenv.axon_hooks glue so trace=True captures NTFF."""
    import types
    if "antenv.axon_hooks" in sys.modules:
        return
    try:
        from trn_agent_boot.trn_boot import _ntff_profile_via_ctypes
        hook = _ntff_profile_via_ctypes("/opt/axon/libaxon_pjrt.so")
        m = types.ModuleType("antenv.axon_hooks")
        m.get_axon_ntff_profile_hook = lambda: hook
        m.set_axon_ntff_profile_hook = lambda h: None
        sys.modules["antenv.axon_hooks"] = m
    except Exception:
        pass
